# revision 1
# baseline (speedup 1.0000x reference)
"""GraphSAGE 2-layer GNN kernel for 8 TRN2 NeuronCores — v2 fallback.

No DMA-accumulate: layer 1 streams a host-packed feature-major bf16 message
table (unit-stride DVE reduce); layer 2 uses batched 64-column indirect
gathers (8K descriptors/instr) into SBUF tiles + strided DVE reduce.
bf16 h table, chunked AllGather overlapped with the layer-1 block phase.
"""
import os
import sys
sys.path.insert(0, '/opt/trn_rl_repo')
import numpy as np
import ml_dtypes

import concourse.bass as bass
import concourse.tile as tile
from concourse import bacc, mybir
from concourse.bass_utils import run_bass_kernel_spmd
from concourse.masks import make_identity

N_CORES = 8
N_NODES = 100000
D = 128
SHARD = N_NODES // N_CORES
CLASSES = [2, 4, 6, 8, 10, 12, 14, 16, 18, 20, 22, 24, 26, 28, 30, 32,
           36, 40, 48, 64, 96, 128]
N_AG = 6
OOB = 1 << 29
MC = 64           # gather/stream group width (columns)

BF16 = ml_dtypes.bfloat16


def _class_of(deg):
    for L in CLASSES:
        if deg <= L:
            return L
    raise AssertionError


def _pack_cores(per_core_dsts_deg):
    """Degree-sorted blocks: per core, sort nodes by degree descending and cut
    into ceil(SHARD/128) blocks of 128; block width = cross-core max degree in
    that block (sorted order keeps within-block spread tiny)."""
    per_core_sorted = [sorted(dd, key=lambda nd: -nd[1])
                       for dd in per_core_dsts_deg]
    nblocks = (SHARD + 127) // 128
    class_of_block = []
    for b in range(nblocks):
        mx = 1
        for ns in per_core_sorted:
            seg = ns[b * 128:(b + 1) * 128]
            if seg:
                mx = max(mx, seg[0][1])     # first entry has the max degree
        assert mx <= MC, f"block degree {mx} exceeds MC={MC}"
        class_of_block.append(mx)

    blocks_per_core = []
    for ns in per_core_sorted:
        blocks = []
        for b in range(nblocks):
            seg = ns[b * 128:(b + 1) * 128]
            nodes = [n for n, _ in seg] + [-1] * (128 - len(seg))
            blocks.append((class_of_block[b], nodes))
        blocks_per_core.append(blocks)

    # MC-aligned column layout: a block never straddles an MC boundary
    cols = []
    cur = 0
    for L in class_of_block:
        if L <= MC and cur % MC + L > MC:
            cur += MC - (cur % MC)
        elif L > MC and cur % MC:
            cur += MC - (cur % MC)
        cols.append(cur)
        cur += L
    m_total = cur + ((-cur) % MC)
    return blocks_per_core, class_of_block, cols, m_total


def _build_tables(x, edge_index):
    src = np.asarray(edge_index[0], dtype=np.int64)
    dst = np.asarray(edge_index[1], dtype=np.int64)
    deg = np.bincount(dst, minlength=N_NODES).astype(np.int64)

    order = np.argsort(dst, kind='stable')
    src_sorted = src[order]
    dst_starts = np.zeros(N_NODES + 1, dtype=np.int64)
    np.cumsum(deg, out=dst_starts[1:])

    per_core_dd = []
    for c in range(N_CORES):
        lo, hi = c * SHARD, (c + 1) * SHARD
        per_core_dd.append([(int(n), int(deg[n])) for n in range(lo, hi)])
    blocks_per_core, class_of_block, cols, m_total = _pack_cores(per_core_dd)

    nseg = len(class_of_block)
    bounds = [round(g * nseg / N_AG) for g in range(N_AG + 1)]
    bounds[N_AG] = nseg + 1          # last chunk also carries the zero block
    ag_sizes = [bounds[g + 1] - bounds[g] for g in range(N_AG)]

    def node_row_of(c, b, p):
        g = next(i for i in range(N_AG) if bounds[i] <= b < bounds[i + 1])
        return (8 * bounds[g] + (ag_sizes[g] * c) + (b - bounds[g])) * 128 + p

    zero_row = node_row_of(0, nseg, 0)   # core-0 zero block, partition 0

    x_bf = np.ascontiguousarray(x.astype(BF16))
    invdeg = 1.0 / np.maximum(deg.astype(np.float32), 1.0)

    node_row = np.full(N_NODES, -1, dtype=np.int64)
    for c in range(N_CORES):
        for b, (_L, nodes) in enumerate(blocks_per_core[c]):
            for p, n in enumerate(nodes):
                if n >= 0:
                    node_row[n] = node_row_of(c, b, p)
    assert (node_row >= 0).all()

    cores = []
    for c in range(N_CORES):
        blocks = blocks_per_core[c]
        slot_src = np.full((128, m_total), -1, dtype=np.int64)
        idx2 = np.full((128, m_total), zero_row, dtype=np.int32)
        inv_tile = np.ones((128, nseg), dtype=np.float32)
        xdT = np.zeros((128, nseg * 128), dtype=BF16)
        node_of = np.full((128, nseg), -1, dtype=np.int64)

        for b, (L, nodes) in enumerate(blocks):
            col = cols[b]
            for p, n in enumerate(nodes):
                node_of[p, b] = n
                if n < 0:
                    continue
                s0, s1 = dst_starts[n], dst_starts[n + 1]
                srcs = src_sorted[s0:s1]
                k = len(srcs)
                slot_src[p, col:col + k] = srcs
                idx2[p, col:col + k] = node_row[srcs]

        # feature-major t1: block region [col, col+L) stores, per partition,
        # element (f*L + l) = x[src_l, f]
        t1 = np.zeros((128, m_total * D), dtype=BF16)
        for b, (L, nodes) in enumerate(blocks):
            col = cols[b]
            ss = slot_src[:, col:col + L]                  # [128, L]
            val = np.where(ss[:, :, None] >= 0,
                           x_bf[np.maximum(ss, 0)], BF16(0))   # [128, L, D]
            t1[:, col * D:(col + L) * D] = \
                val.transpose(0, 2, 1).reshape(128, L * D)

        nv = node_of >= 0
        pp, bb = np.nonzero(nv)
        nodes_v = node_of[pp, bb]
        inv_tile[pp, bb] = invdeg[nodes_v]
        xdT[:, bb * 128 + pp] = x_bf[nodes_v].T

        cores.append({"t1": t1, "idx2": idx2, "inv": inv_tile,
                      "xdT": xdT, "node_of": node_of})

    meta = {"nseg": nseg, "m": m_total, "cols": cols,
            "class_of_block": class_of_block, "bounds": bounds,
            "ag_sizes": ag_sizes, "cores": cores}
    return meta


def _build_program(meta):
    nseg, m = meta["nseg"], meta["m"]
    cols, cls = meta["cols"], meta["class_of_block"]
    bounds = meta["bounds"]
    full_rows = 8 * (nseg + 1) * 128
    n_groups = m // MC

    # blocks of each MC group
    group_blocks = [[] for _ in range(n_groups)]
    for b in range(nseg):
        L = cls[b]
        g0, g1 = cols[b] // MC, (cols[b] + L - 1) // MC
        for g in range(g0, g1 + 1):
            group_blocks[g].append(b)

    f32, bf16, i32 = mybir.dt.float32, mybir.dt.bfloat16, mybir.dt.int32
    nc = bacc.Bacc("TRN2", target_bir_lowering=False, debug=False,
                   num_devices=N_CORES)

    t1_d = nc.dram_tensor("t1", [128, m * D], bf16, kind="ExternalInput")
    idx2_d = nc.dram_tensor("idx2", [128, m], i32, kind="ExternalInput")
    inv_d = nc.dram_tensor("inv", [128, nseg], f32, kind="ExternalInput")
    xdT_d = nc.dram_tensor("xdT", [128, nseg * 128], bf16, kind="ExternalInput")
    w1l_d = nc.dram_tensor("w1lT", [128, 128], bf16, kind="ExternalInput")
    w1r_d = nc.dram_tensor("w1rT", [128, 128], bf16, kind="ExternalInput")
    w2l_d = nc.dram_tensor("w2lT", [128, 128], bf16, kind="ExternalInput")
    w2r_d = nc.dram_tensor("w2rT", [128, 128], bf16, kind="ExternalInput")
    b1_d = nc.dram_tensor("b1", [128, 1], f32, kind="ExternalInput")
    b2_d = nc.dram_tensor("b2", [128, 1], f32, kind="ExternalInput")
    outT_d = nc.dram_tensor("outT", [128, nseg * 128], f32, kind="ExternalOutput")

    h_shard = nc.dram_tensor("h_shard", [(nseg + 1) * 128, D], f32)
    h_full = nc.dram_tensor("h_full", [full_rows, D], f32,
                            addr_space="Shared")

    with tile.TileContext(nc) as tc:
        with (
            tc.tile_pool(name="msg", bufs=2) as mp,
            tc.tile_pool(name="persist", bufs=1) as pp,
            tc.tile_pool(name="work", bufs=3) as wp,
            tc.tile_pool(name="psum", bufs=2, space="PSUM") as psp,
        ):
            agg = pp.tile([128, nseg, D], f32, tag="agg")
            hT = pp.tile([128, nseg * 128], bf16, tag="hT")
            inv_t = pp.tile([128, nseg], f32, tag="inv")
            nc.sync.dma_start(out=inv_t[:], in_=inv_d.ap())
            idx2_t = pp.tile([128, m], i32, tag="idx2")
            nc.sync.dma_start(out=idx2_t[:], in_=idx2_d.ap())
            w1l = pp.tile([128, 128], bf16, tag="w1l")
            nc.sync.dma_start(out=w1l[:], in_=w1l_d.ap())
            w1r = pp.tile([128, 128], bf16, tag="w1r")
            nc.sync.dma_start(out=w1r[:], in_=w1r_d.ap())
            w2l = pp.tile([128, 128], bf16, tag="w2l")
            nc.sync.dma_start(out=w2l[:], in_=w2l_d.ap())
            w2r = pp.tile([128, 128], bf16, tag="w2r")
            nc.sync.dma_start(out=w2r[:], in_=w2r_d.ap())
            b1_t = pp.tile([128, 1], f32, tag="b1")
            nc.sync.dma_start(out=b1_t[:], in_=b1_d.ap())
            b2_t = pp.tile([128, 1], f32, tag="b2")
            nc.sync.dma_start(out=b2_t[:], in_=b2_d.ap())
            ident = pp.tile([128, 128], f32, tag="ident")
            make_identity(nc, ident[:])
            ident_bf = pp.tile([128, 128], bf16, tag="identbf")
            nc.scalar.copy(ident_bf[:], ident[:])

            # zero block for layer-2 padding gathers (last AG chunk)
            zt = wp.tile([128, 128], f32, tag="zero")
            nc.vector.memset(zt[:], 0.0)
            nc.sync.dma_start(out=h_shard.ap()[nseg * 128:(nseg + 1) * 128, :],
                              in_=zt[:])

            def block_phase(b, wl, wr, bias_t, rhs, func, out_fn):
                mean_b = wp.tile([128, 128], f32, tag="mean")
                nc.vector.tensor_scalar_mul(
                    mean_b[:], agg[:, b, :], inv_t[:, b:b + 1])
                mT_ps = psp.tile([128, 128], f32, space="PSUM", tag="tp")
                nc.tensor.transpose(out=mT_ps[:], in_=mean_b[:],
                                    identity=ident[:])
                meanT = wp.tile([128, 128], bf16, tag="meanT")
                nc.scalar.copy(meanT[:], mT_ps[:])
                ps = psp.tile([128, 128], f32, space="PSUM", tag="mm")
                nc.tensor.matmul(out=ps[:], lhsT=wl[:], rhs=meanT[:],
                                 start=True, stop=False)
                nc.tensor.matmul(out=ps[:], lhsT=wr[:], rhs=rhs,
                                 start=False, stop=True)
                out_fn(b, ps, bias_t, func)

            # ---------------- layer 1: stream feature-major t1 ----------
            for g in range(n_groups):
                t = mp.tile([128, MC * D], bf16, tag="msg")
                nc.sync.dma_start(out=t[:],
                                  in_=t1_d.ap()[:, g * MC * D:(g + 1) * MC * D])
                for b in group_blocks[g]:
                    L = cls[b]
                    lc = cols[b] - g * MC
                    view = t[:, lc * D:(lc + L) * D].rearrange(
                        "p (f l) -> p f l", f=D, l=L)
                    nc.vector.tensor_reduce(
                        out=agg[:, b, :], in_=view,
                        axis=mybir.AxisListType.X, op=mybir.AluOpType.add)

            def l1_rhs(b):
                xT_b = wp.tile([128, 128], bf16, tag="xTb")
                nc.sync.dma_start(out=xT_b[:],
                                  in_=xdT_d.ap()[:, b * 128:(b + 1) * 128])
                return xT_b[:]

            def l1_out(b, ps, bias_t, func):
                nc.scalar.activation(out=hT[:, b * 128:(b + 1) * 128],
                                     in_=ps[:], func=func, bias=bias_t[:],
                                     scale=1.0)
                hps = psp.tile([128, 128], bf16, space="PSUM", tag="tp2")
                nc.tensor.transpose(out=hps[:],
                                    in_=hT[:, b * 128:(b + 1) * 128],
                                    identity=ident_bf[:])
                h_blk = wp.tile([128, 128], f32, tag="hblk")
                nc.scalar.copy(h_blk[:], hps[:])
                nc.sync.dma_start(out=h_shard.ap()[b * 128:(b + 1) * 128, :],
                                  in_=h_blk[:])

            for b in range(nseg):
                block_phase(b, w1l, w1r, b1_t, l1_rhs(b),
                            mybir.ActivationFunctionType.Relu, l1_out)
                g = next((i for i in range(N_AG)
                          if bounds[i + 1] in (b + 1, b + 2) and
                          (bounds[i + 1] == b + 1 or b + 1 == nseg)), None)
                if g is not None:
                    s, e = bounds[g], bounds[g + 1]
                    nc.gpsimd.collective_compute(
                        "AllGather", mybir.AluOpType.bypass,
                        ins=[h_shard.ap()[s * 128:e * 128, :].opt()],
                        outs=[h_full.ap()[8 * s * 128:8 * e * 128, :].opt()],
                        replica_groups=[list(range(N_CORES))],
                    )

            # ---------------- layer 2: batched gather + strided reduce ---
            # per-column indirect gathers (the only correct+fast form of the
            # generic SWDGE indirect path); pad slots fetch the zero row.
            # Columns in alignment gaps (no block) are skipped entirely.
            for g in range(n_groups):
                t = mp.tile([128, MC, D], f32, tag="msg2")
                for b in group_blocks[g]:
                    L = cls[b]
                    lc = cols[b] - g * MC
                    for j in range(L):
                        nc.gpsimd.indirect_dma_start(
                            out=t[:, lc + j, :], out_offset=None,
                            in_=h_full.ap(),
                            in_offset=bass.IndirectOffsetOnAxis(
                                ap=idx2_t[:, cols[b] + j:cols[b] + j + 1],
                                axis=0))
                for b in group_blocks[g]:
                    L = cls[b]
                    lc = cols[b] - g * MC
                    view = t[:, lc:lc + L, :].rearrange(
                        "p a b -> p (a b)").rearrange(
                        "p (l f) -> p f l", l=L, f=D)
                    nc.vector.tensor_reduce(
                        out=agg[:, b, :], in_=view,
                        axis=mybir.AxisListType.X, op=mybir.AluOpType.add)

            def l2_out(b, ps, bias_t, func):
                oT = wp.tile([128, 128], f32, tag="oT")
                nc.scalar.activation(out=oT[:], in_=ps[:], func=func,
                                     bias=bias_t[:], scale=1.0)
                nc.sync.dma_start(out=outT_d.ap()[:, b * 128:(b + 1) * 128],
                                  in_=oT[:])

            for b in range(nseg):
                block_phase(b, w2l, w2r, b2_t, hT[:, b * 128:(b + 1) * 128],
                            mybir.ActivationFunctionType.Identity, l2_out)

    nc.compile()
    return nc


_CACHE = {}
LAST_RESULTS = None


def kernel(x, edge_index, W1_l, b1_l, W1_r, W2_l, b2_l, W2_r):
    global LAST_RESULTS
    x = np.asarray(x, dtype=np.float32)
    meta = _build_tables(x, np.asarray(edge_index))

    key = (meta["nseg"], meta["m"])
    if key not in _CACHE:
        _CACHE[key] = _build_program(meta)
    nc = _CACHE[key]

    in_maps = []
    for c in range(N_CORES):
        ci = meta["cores"][c]
        in_maps.append({
            "t1": ci["t1"], "idx2": ci["idx2"], "inv": ci["inv"],
            "xdT": ci["xdT"],
            "w1lT": np.asarray(W1_l, np.float32).T.astype(BF16).copy(),
            "w1rT": np.asarray(W1_r, np.float32).T.astype(BF16).copy(),
            "w2lT": np.asarray(W2_l, np.float32).T.astype(BF16).copy(),
            "w2rT": np.asarray(W2_r, np.float32).T.astype(BF16).copy(),
            "b1": np.asarray(b1_l, np.float32).reshape(128, 1).copy(),
            "b2": np.asarray(b2_l, np.float32).reshape(128, 1).copy(),
        })

    res = run_bass_kernel_spmd(nc, in_maps, core_ids=list(range(N_CORES)),
                               tmpdir=os.environ.get("KERNEL_TRACE_DIR"))
    LAST_RESULTS = res

    out = np.zeros((N_NODES, D), dtype=np.float32)
    for c in range(N_CORES):
        outT = res.results[c]["outT"].reshape(128, meta["nseg"] * 128)
        node_of = meta["cores"][c]["node_of"]
        pp_, bb = np.nonzero(node_of >= 0)
        nodes = node_of[pp_, bb]
        out[nodes] = outT[:, bb * 128 + pp_].T
    return out

